# revision 14
# baseline (speedup 1.0000x reference)
"""Trainium2 Bass kernel for MoE routing (2-layer expert MLP + softmax).

Strategy: expert-parallel across the 8 NeuronCores. The reference computes
all 8 experts for every sample and then gathers the one selected by
`domain`; mathematically only the selected expert's MLP matters per sample.
The host groups samples by expert (argsort of `domain`), core e receives
only the ~B/8 samples routed to expert e (padded to a uniform per-core
capacity so all cores run the same SPMD program) plus expert e's weights.
Each core runs a dense 2-layer MLP + softmax in a transposed layout:

    hT[f2, n]  = relu(W1[:, f2].T @ xT[:, n] + b1[f2])   (PE fp32r + ACT)
    lT[c, n]   = W2[:, c].T @ hT[:, n]                   (PE fp32r)
    expT       = exp(lT + b2)                            (ACT)
    sT[c, n]   = ones[C,C].T @ expT                      (PE: partition sum,
                                                          pre-broadcast to C)
    out[c, n]  = expT * (1 / sT)                         (DVE)

Matmuls run as float32r (1 cycle/row on the PE when the moving dim is
>=256, vs 4 for plain fp32); the PE rounds fp32 operands internally, so
the fp32 input bits are DMA'd unmodified into tiles declared float32r.
Inputs are pre-arranged on the host into the exact SBUF tile layouts
([partition, k, n] blocks) so every DMA descriptor is one long contiguous
run per partition. The host scatters each core's [C, cap] output back to
the original row order. All heavy data movement and FLOPs run on device;
the host only computes routing indices and re-layouts.
"""

import math

import numpy as np

import concourse.bacc as bacc
import concourse.bass as bass
import concourse.mybir as mybir
import concourse.tile as tile
from concourse.bass import ds
from concourse.bass_utils import run_bass_kernel_spmd

N_CORES = 8

_program_cache: dict[tuple, object] = {}


def _chunk_sizes(cap: int) -> list[int]:
    """Split cap (multiple of 128) into matmul chunks, all >=256 wide so
    float32r matmuls run at the 1-cycle/row rate."""
    q, r = divmod(cap, 512)
    chunks = [512] * q
    if r == 128:
        if q:
            chunks = chunks[:-1] + [384, 256]
        else:
            chunks = [128]
    elif r:
        chunks.append(r)
    return chunks


def _build_program(cap: int, F1: int, F2: int, C: int):
    """Build the per-core SPMD bass program for a dense [cap, F1] -> [C, cap]
    expert MLP in transposed layout."""
    key = (cap, F1, F2, C)
    if key in _program_cache:
        return _program_cache[key]

    assert F1 % 128 == 0 and F2 % 128 == 0 and cap % 128 == 0
    K1 = F1 // 128  # contraction tiles for layer 1
    M1 = F2 // 128  # output partition tiles for layer 1
    K2 = F2 // 128  # contraction tiles for layer 2
    assert C <= 128

    f32 = mybir.dt.float32
    f32r = mybir.dt.float32r
    nc = bacc.Bacc(None, target_bir_lowering=False, debug=False)

    chunks = _chunk_sizes(cap)

    # All inputs arrive pre-arranged in SBUF tile layout.
    x_d = [
        nc.dram_tensor(f"xt{ci}", [128, K1, cn], f32r, kind="ExternalInput")
        for ci, cn in enumerate(chunks)
    ]
    w1_d = nc.dram_tensor("w1", [128, K1, F2], f32r, kind="ExternalInput")
    b1_d = nc.dram_tensor("b1t", [128, M1], f32, kind="ExternalInput")
    w2_d = nc.dram_tensor("w2", [128, K2, C], f32r, kind="ExternalInput")
    b2_d = nc.dram_tensor("b2t", [C, 1], f32, kind="ExternalInput")
    out_d = nc.dram_tensor("outT", [C, cap], f32, kind="ExternalOutput")

    with tile.TileContext(nc) as tc:
        with (
            tc.tile_pool(name="const", bufs=1) as const_pool,
            tc.tile_pool(name="xin", bufs=len(chunks)) as x_pool,
            tc.tile_pool(name="h", bufs=2 * M1) as h_pool,
            tc.tile_pool(name="exp", bufs=3) as e_pool,
            tc.tile_pool(name="out", bufs=2) as o_pool,
            tc.tile_pool(name="rec", bufs=2) as r_pool,
            tc.tile_pool(name="ph", bufs=4, space="PSUM") as ph_pool,
            tc.tile_pool(name="pl", bufs=2, space="PSUM") as pl_pool,
            tc.tile_pool(name="pb", bufs=2, space="PSUM") as pb_pool,
        ):
            # Weights on the ACT HWDGE ring (parallel to the x stream on the
            # SP ring). w1 is split per k-tile so the first layer-1 matmul
            # only waits for one 256KB slice, not the whole 2MB.
            w1_sb = const_pool.tile([128, K1, F2], f32r)
            for k in range(K1):
                nc.scalar.dma_start(w1_sb[:, k, :], w1_d[:, k, :])
            b1_sb = const_pool.tile([128, M1], f32)
            nc.scalar.dma_start(b1_sb[:], b1_d[:])
            w2_sb = const_pool.tile([128, K2, C], f32r)
            nc.scalar.dma_start(w2_sb[:], w2_d[:])
            b2_sb = const_pool.tile([C, 1], f32)
            nc.scalar.dma_start(b2_sb[:], b2_d[:])

            # ones[C, C]: a single matmul against this computes the
            # partition-dim sum of exp AND broadcasts it back to all C
            # partitions in one shot. (memset can't write f32r; round via a
            # DVE copy.)
            ones_f32 = const_pool.tile([C, C], f32)
            nc.gpsimd.memset(ones_f32[:], 1.0)
            ones_cc = const_pool.tile([C, C], f32r)
            nc.vector.tensor_copy(ones_cc[:], ones_f32[:])

            def body(ci: int, cn: int):
                """Layer1 + relu + layer2 + exp for one batch chunk."""
                xt = x_pool.tile([128, K1, cn], f32r, tag="xt")
                if ci == 0:
                    # Split per k so PE can start after the first slice.
                    for k in range(K1):
                        nc.sync.dma_start(xt[:, k, :], x_d[ci][:, k, :])
                else:
                    nc.sync.dma_start(xt[:], x_d[ci][:])

                ht = []
                for m in range(M1):
                    ph = ph_pool.tile([128, cn], f32, tag="ph")
                    for k in range(K1):
                        nc.tensor.matmul(
                            ph[:],
                            w1_sb[:, k, ds(m * 128, 128)],
                            xt[:, k, :],
                            start=(k == 0),
                            stop=(k == K1 - 1),
                        )
                    hm = h_pool.tile([128, cn], f32r, tag="ht")
                    nc.scalar.activation(
                        hm[:],
                        ph[:],
                        mybir.ActivationFunctionType.Relu,
                        bias=b1_sb[:, ds(m, 1)],
                    )
                    ht.append(hm)

                pl = pl_pool.tile([C, cn], f32, tag="pl")
                for k in range(K2):
                    nc.tensor.matmul(
                        pl[:],
                        w2_sb[:, k, :],
                        ht[k][:],
                        start=(k == 0),
                        stop=(k == K2 - 1),
                    )
                expt = e_pool.tile([C, cn], f32r, tag="expt")
                nc.scalar.activation(
                    expt[:],
                    pl[:],
                    mybir.ActivationFunctionType.Exp,
                    bias=b2_sb[:, 0:1],
                )
                return expt

            def tail(expt, n0: int, cn: int):
                """Softmax normalization + store for one chunk."""
                pb = pb_pool.tile([C, cn], f32, tag="pb")
                nc.tensor.matmul(pb[:], ones_cc[:], expt[:], start=True, stop=True)
                rec = r_pool.tile([C, cn], f32, tag="rec")
                nc.vector.reciprocal_approx_fast(rec[:], pb[:])
                ot = o_pool.tile([C, cn], f32, tag="ot")
                nc.vector.tensor_mul(ot[:], expt[:].bitcast(f32), rec[:])
                nc.sync.dma_start(out_d[:, ds(n0, cn)], ot[:])

            # Emit tails one chunk behind the bodies so the PE stream stays
            # dense and softmax tails overlap the next chunk's matmuls.
            pending = None
            n0 = 0
            for ci, cn in enumerate(chunks):
                expt = body(ci, cn)
                if pending is not None:
                    tail(*pending)
                pending = (expt, n0, cn)
                n0 += cn
            tail(*pending)

    nc.compile()
    _program_cache[key] = nc
    return nc


def kernel(domain, x, W1, b1, W2, b2):
    domain = np.asarray(domain)
    x = np.ascontiguousarray(np.asarray(x, dtype=np.float32))
    W1 = np.asarray(W1, dtype=np.float32)
    b1 = np.asarray(b1, dtype=np.float32)
    W2 = np.asarray(W2, dtype=np.float32)
    b2 = np.asarray(b2, dtype=np.float32)

    B, F1 = x.shape
    E, _, F2 = W1.shape
    C = W2.shape[2]
    K1 = F1 // 128
    K2 = F2 // 128
    assert E == N_CORES

    idx = [np.nonzero(domain == e)[0] for e in range(E)]
    counts = [len(i) for i in idx]
    cap = max(512, int(math.ceil(max(counts) / 128)) * 128)
    chunks = _chunk_sizes(cap)

    nc = _build_program(cap, F1, F2, C)

    in_maps = []
    for e in range(E):
        xT = np.zeros((F1, cap), np.float32)
        xT[:, : counts[e]] = x[idx[e]].T
        # [F1, cap] -> per-chunk [128, K1, cn] blocks (SBUF tile layout).
        xT3 = xT.reshape(K1, 128, cap)
        m = {
            "w1": np.ascontiguousarray(
                W1[e].reshape(K1, 128, F2).transpose(1, 0, 2)
            ),
            "b1t": np.ascontiguousarray(b1[e].reshape(F2 // 128, 128).T),
            "w2": np.ascontiguousarray(
                W2[e].reshape(K2, 128, C).transpose(1, 0, 2)
            ),
            "b2t": np.ascontiguousarray(b2[e].reshape(C, 1)),
        }
        n0 = 0
        for ci, cn in enumerate(chunks):
            m[f"xt{ci}"] = np.ascontiguousarray(
                xT3[:, :, n0 : n0 + cn].transpose(1, 0, 2)
            )
            n0 += cn
        in_maps.append(m)

    res = run_bass_kernel_spmd(nc, in_maps, core_ids=list(range(N_CORES)))

    out = np.empty((B, C), np.float32)
    for e in range(E):
        out[idx[e]] = res.results[e]["outT"][:, : counts[e]].T
    return out
